# revision 20
# baseline (speedup 1.0000x reference)
"""Trainium2 Bass kernel for nn_DepthWiseConv_AConnect (depthwise 3x3 conv with
per-pool multiplicative weight/bias noise, followed by 8-bit LQuant).

Strategy (8 NeuronCores, data-parallel over the pool axis):
  - Core p handles pool group p: 8 images, Werr[p], Berr[p].
  - The conv runs as accumulating DIAGONAL matmuls on the TensorEngine in
    fp8-e4m3 DoubleRow mode (0.5 cycles/column, two contraction planes per
    instruction).  fp8's 3-bit mantissa alone would fail the rel-err gate,
    so both operands are hi/lo split:
      x  = x_hi + x_lo   (each fp8; residual ~0.075% rms)
      w  = w8  + r8      (each fp8)
    and each PSUM bank accumulates 14 DoubleRow matmuls:
      9 "A" pairs:  w8_t*x_hi_t + w8_t*x_lo_t          (t = 0..8)
      3 vertical "B" pairs:  r8_t*x_hi_t + r8_{t+3}*x_hi_{t+3}   (t = 0,1,2)
      1 horizontal "B" pair: r8_6*x_hi_6 + r8_7*x_hi_7
      1 padded "B":          r8_8*x_hi_8 + 0
    The only dropped term is r8*x_lo (~0.07% of each tap product).
    B-pair rhs access patterns need two shifts inside one AP, so the host
    ships two extra pre-shifted copies of the hi plane (shift 56 = one row,
    and shift 1); plane pairs are then plain strided views
    ([0:2], [0::2], [0::3]) of one [128, 4, H*W] SBUF tile.
    14 x 243 = 3402 PE cycles per bank vs 9 x 486 = 4374 for fp16.
  - PSUM evict IS the quantization: activation/tensor_scalar compute
    (psum + 127*bias_noisy[c]) with an int8 output dtype.  The hardware
    f32->int8 convert rounds to nearest-even (verified bit-identical to the
    jnp.round magic-constant trick) and saturates to [-128, 127]; the host
    maps the (astronomically rare) -128 back to -127 to match the
    reference's clip-before-scale.  ScalarE drains 2 banks per half and
    VectorE the third, so the post-matmul drain chain is short.
  - Host divides by 127 and transposes back to NHWC.

Performance notes (measured on trn2, core 0):
  - fp16 version of this kernel (9 matmuls/bank) ran 199 us with the PE
    gap-free at full p-state; columns = out_pixels x taps x C/128 is
    provably minimal for a depthwise conv on the 128-partition PE, so the
    only way down was fp8 DoubleRow's 0.5 cycles/column.
  - GPSIMD must not touch the data path: a [128, 2916] tensor_scalar costs
    ~43 us there (vs ~1.7 us on VectorE), which made the original version
    DMA-starved and 3.5x slower.
  - Head ~13 us (framework preamble + first loads), tail ~5 us (drain +
    epilogue); run-to-run DVFS variance is ~20%.
"""
import sys

import numpy as np

try:
    import concourse.bacc as bacc_mod
except ImportError:
    sys.path.insert(0, "/opt/trn_rl_repo")
    import concourse.bacc as bacc_mod

import concourse.mybir as mybir
from concourse.tile import TileContext
from concourse.bass_utils import run_bass_kernel_spmd
from contextlib import ExitStack

POOL = 8
NB = 8            # images per pool group (64 / 8)
H = W = 56
HO = WO = 54
C = 256
NCH = 2           # channel chunks of 128
NPIX = H * W      # 3136
NOUT = HO * WO    # 2916
NBANK = 6         # psum bank tiles per plane (6 * 486 = 2916)
BANKN = 486       # output pixels per psum tile (9 rows x 54)
NMM = 14          # DoubleRow matmuls per bank (9 A-pairs + 5 B-pairs)
S = 127.0

f32 = mybir.dt.float32
f8 = mybir.dt.float8e4
i8 = mybir.dt.int8
Alu = mybir.AluOpType
Act = mybir.ActivationFunctionType
DR = mybir.MatmulPerfMode.DoubleRow

_cached = {}


def _build():
    nc = bacc_mod.Bacc()
    xt = nc.dram_tensor("xt", [NB, NCH, 128, 4, NPIX], f8, kind="ExternalInput")
    wdg = nc.dram_tensor("wdg", [128, NCH, NMM, 2, 128], f8,
                         kind="ExternalInput")
    bv = nc.dram_tensor("bv", [128, NCH, 1], f32, kind="ExternalInput")
    out = nc.dram_tensor("out", [NB, NCH, 128, NOUT], i8, kind="ExternalOutput")

    with TileContext(nc) as tc, ExitStack() as ctx:
        consts = ctx.enter_context(tc.tile_pool(name="consts", bufs=1))
        xpool = ctx.enter_context(tc.tile_pool(name="xpool", bufs=3))
        opool = ctx.enter_context(tc.tile_pool(name="opool", bufs=2))
        pspool = ctx.enter_context(tc.tile_pool(name="pspool", bufs=2, space="PSUM"))

        ws = consts.tile([128, NCH, NMM, 2, 128], f8)
        nc.sync.dma_start(out=ws, in_=wdg[:])
        bt = consts.tile([128, NCH, 1], f32)

        for n in range(NB):
            for q in range(NCH):
                xs = xpool.tile([128, 4, NPIX], f8, tag="xs")
                # planes 0,1 (x_hi, x_lo) gate the A-pair matmuls; shifted
                # planes 2,3 only gate the B-pairs
                nc.sync.dma_start(out=xs[:, 0:2], in_=xt[n, q, :, 0:2])
                nc.sync.dma_start(out=xs[:, 2:4], in_=xt[n, q, :, 2:4])
                if n == 0 and q == 0:
                    # bias load is off the first-matmul critical path
                    nc.sync.dma_start(out=bt, in_=bv[:])
                xv = xs.rearrange("p f (h w) -> p f h w", w=W)
                ot = opool.tile([128, NOUT], i8, tag="ot")
                HN = NOUT // 2
                for half in range(2):
                    banks = range(3 * half, 3 * half + 3)
                    pss = [pspool.tile([128, BANKN], f32, tag=f"ps{i}",
                                       name=f"ps{i}")
                           for i in range(3)]
                    for m in range(NMM):
                        if m < 9:        # A: w8_t*x_hi + w8_t*x_lo
                            i, j = divmod(m, 3)
                            pl = slice(0, 2)
                        elif m < 12:     # B vert: taps (m-9, m-6), planes 0,2
                            i, j = 0, m - 9
                            pl = slice(0, 4, 2)
                        elif m == 12:    # B horiz: taps (6,7), planes 0,3
                            i, j = 2, 0
                            pl = slice(0, 4, 3)
                        else:            # tap 8 alone; plane-1 weights are 0
                            i, j = 2, 2
                            pl = slice(0, 2)
                        for bi, b in enumerate(banks):
                            rhs = xv[:, pl, 9 * b + i: 9 * b + i + 9, j: j + 54]
                            nc.tensor.matmul(pss[bi], lhsT=ws[:, q, m],
                                             rhs=rhs, start=(m == 0),
                                             stop=(m == NMM - 1),
                                             perf_mode=DR,
                                             skip_group_check=True)
                    for bi, b in enumerate(banks):
                        osl = ot[:, BANKN * b: BANKN * (b + 1)]
                        if bi < 2:
                            nc.scalar.activation(out=osl, in_=pss[bi],
                                                 func=Act.Identity,
                                                 bias=bt[:, q], scale=1.0)
                        else:
                            nc.vector.tensor_scalar(out=osl, in0=pss[bi],
                                                    scalar1=bt[:, q],
                                                    scalar2=None, op0=Alu.add)
                    hs = slice(HN * half, HN * (half + 1))
                    nc.sync.dma_start(out=out[n, q, :, hs], in_=ot[:, hs])

    nc.finalize()
    return nc


def kernel(X, W, bias, Werr, Berr, _trace=False):
    X = np.asarray(X, np.float32)
    W = np.asarray(W, np.float32)
    bias = np.asarray(bias, np.float32)
    Werr = np.asarray(Werr, np.float32)
    Berr = np.asarray(Berr, np.float32)

    if "nc" not in _cached:
        _cached["nc"] = _build()
    nc = _cached["nc"]

    npf8 = mybir.dt.np(f8)
    w3 = W[..., 0]             # [3, 3, 256]
    we3 = Werr[..., 0]         # [8, 3, 3, 256]
    idx = np.arange(128)

    in_maps = []
    for p in range(POOL):
        xp = X[p * NB:(p + 1) * NB].reshape(NB, NPIX, C)
        xp = np.ascontiguousarray(xp.transpose(0, 2, 1)).reshape(
            NB, NCH, 128, NPIX)
        xhi8 = xp.astype(npf8)
        xlo8 = (xp - xhi8.astype(np.float32)).astype(npf8)
        pl2 = np.zeros_like(xhi8)           # x_hi shifted one row
        pl2[..., :NPIX - H] = xhi8[..., H:]
        pl3 = np.zeros_like(xhi8)           # x_hi shifted one column
        pl3[..., :NPIX - 1] = xhi8[..., 1:]
        xq = np.stack([xhi8, xlo8, pl2, pl3], axis=3)  # [NB,NCH,128,4,NPIX]

        w_eff = (np.float32(S) * w3 * we3[p]).reshape(9, C)  # fp32 [9, 256]
        w8 = w_eff.astype(npf8)
        r8 = (w_eff - w8.astype(np.float32)).astype(npf8)
        wdg = np.zeros((128, NCH, NMM, 2, 128), npf8)
        for q in range(NCH):
            w8q = w8[:, 128 * q:128 * (q + 1)]  # [9, 128]
            r8q = r8[:, 128 * q:128 * (q + 1)]
            for t in range(9):
                wdg[idx, q, t, 0, idx] = w8q[t]
                wdg[idx, q, t, 1, idx] = w8q[t]
            for pk in range(3):
                wdg[idx, q, 9 + pk, 0, idx] = r8q[pk]
                wdg[idx, q, 9 + pk, 1, idx] = r8q[pk + 3]
            wdg[idx, q, 12, 0, idx] = r8q[6]
            wdg[idx, q, 12, 1, idx] = r8q[7]
            wdg[idx, q, 13, 0, idx] = r8q[8]

        b_eff = (np.float32(S) * bias * Berr[p]).astype(np.float32)
        bvv = np.ascontiguousarray(
            b_eff.reshape(NCH, 128, 1).transpose(1, 0, 2))
        in_maps.append({"xt": xq, "wdg": wdg, "bv": bvv})

    res = run_bass_kernel_spmd(nc, in_maps, core_ids=list(range(POOL)),
                               trace=_trace)
    if _trace:
        _cached["last_result"] = res

    outs = []
    for p in range(POOL):
        o = res.results[p]["out"]  # [NB, NCH, 128, NOUT] int8
        o = np.where(o == -128, np.int8(-127), o).astype(np.float32)
        o = o / np.float32(S)
        o = o.reshape(NB, C, HO, WO).transpose(0, 2, 3, 1)  # NHWC
        outs.append(o)
    return np.ascontiguousarray(np.concatenate(outs, axis=0).astype(np.float32))


# revision 22
# speedup vs baseline: 1.5057x; 1.5057x over previous
"""Trainium2 Bass kernel for nn_DepthWiseConv_AConnect (depthwise 3x3 conv with
per-pool multiplicative weight/bias noise, followed by 8-bit LQuant).

Strategy (8 NeuronCores, data-parallel over the pool axis):
  - Core p handles pool group p: 8 images, Werr[p], Berr[p].
  - Inputs ship as fp16 (products of two 11-bit-mantissa halves are exact in
    the fp32 PSUM accumulator); quantized outputs ship as int8.
  - Host pre-transposes X to channels-major [n, c, h*w] fp16.
  - On device, the depthwise conv is 9 accumulating diagonal matmuls on the
    TensorEngine per PSUM bank tile of 486 output pixels (9 output rows):
    psum[c, pix] += diag(127*w_t)[c] * X^T[c, pix + shift_t].
    Matmuls run tap-outer over 3-bank halves so consecutive instructions
    share the stationary weights.
  - PSUM evict IS the quantization: activation/tensor_scalar compute
    (psum + 127*bias_noisy[c]) with an int8 output dtype.  The hardware
    f32->int8 convert rounds to nearest-even (verified bit-identical to the
    jnp.round magic-constant trick) and saturates to [-128, 127]; the host
    maps the (astronomically rare) -128 back to -127 to match the
    reference's clip-before-scale.  ScalarE drains 2 banks per half and
    VectorE the third, so the post-matmul drain chain is short.
  - Host divides by 127 and transposes back to NHWC.

Performance notes (measured on trn2, core 0):
  - The TensorEngine is the critical path and runs gap-free at full p-state:
    864 matmuls x 486 columns x 0.4167 ns ~= 175 us; LDWEIGHTS pipelines
    behind the column stream.  Columns = out_pixels x 9 taps x C/128 is
    provably minimal for a depthwise conv on the 128-partition PE.  fp8
    DoubleRow was measured to give NO speedup for tap-packing: hardware
    streams the doubled moving rows at 0.5 cyc each (same wall time as one
    fp16 matmul), so a hi/lo-split DR variant ran 301 us despite passing
    the rel-err gate at 6.7e-3.
  - GPSIMD must not touch the data path: a [128, 2916] tensor_scalar costs
    ~43 us there (vs ~1.7 us on VectorE), which made the original version
    DMA-starved and 3.5x slower.
  - The MAGIC bias must be added to fp32 data on-chip; folding it into the
    host-side bias rounds the bias to integers (ulp(1.5*2^23) = 1).
  - Head ~13 us (framework preamble + first loads), tail ~9 us (drain +
    epilogue); run-to-run DVFS variance is ~20%.
"""
import sys

import numpy as np

try:
    import concourse.bacc as bacc_mod
except ImportError:
    sys.path.insert(0, "/opt/trn_rl_repo")
    import concourse.bacc as bacc_mod

import concourse.mybir as mybir
from concourse.tile import TileContext
from concourse.bass_utils import run_bass_kernel_spmd
from contextlib import ExitStack

POOL = 8
NB = 8            # images per pool group (64 / 8)
H = W = 56
HO = WO = 54
C = 256
NCH = 2           # channel chunks of 128
NPIX = H * W      # 3136
NOUT = HO * WO    # 2916
NBANK = 6         # psum bank tiles per plane (6 * 486 = 2916)
BANKN = 486       # output pixels per psum tile (9 rows x 54)
MAGIC = 12582912.0  # 1.5 * 2^23
S = 127.0

f32 = mybir.dt.float32
f16 = mybir.dt.float16
i8 = mybir.dt.int8
Alu = mybir.AluOpType
Act = mybir.ActivationFunctionType

_cached = {}


def _build():
    nc = bacc_mod.Bacc()
    xt = nc.dram_tensor("xt", [NB, NCH, 128, NPIX], f16, kind="ExternalInput")
    wdg = nc.dram_tensor("wdg", [128, NCH, 9, 128], f16, kind="ExternalInput")
    bv = nc.dram_tensor("bv", [128, NCH, 1], f32, kind="ExternalInput")
    out = nc.dram_tensor("out", [NB, NCH, 128, NOUT], i8, kind="ExternalOutput")

    with TileContext(nc) as tc, ExitStack() as ctx:
        consts = ctx.enter_context(tc.tile_pool(name="consts", bufs=1))
        xpool = ctx.enter_context(tc.tile_pool(name="xpool", bufs=3))
        tpool = ctx.enter_context(tc.tile_pool(name="tpool", bufs=2))
        opool = ctx.enter_context(tc.tile_pool(name="opool", bufs=2))
        pspool = ctx.enter_context(tc.tile_pool(name="pspool", bufs=2, space="PSUM"))

        ws = consts.tile([128, NCH, 9, 128], f16)
        nc.sync.dma_start(out=ws, in_=wdg[:])
        bt = consts.tile([128, NCH, 1], f32)

        for n in range(NB):
            for q in range(NCH):
                xs = xpool.tile([128, NPIX], f16, tag="xs")
                # split the load so the first half's matmuls (input rows
                # 0..34) don't wait for the whole image
                nc.sync.dma_start(out=xs[:, :35 * W], in_=xt[n, q, :, :35 * W])
                nc.sync.dma_start(out=xs[:, 35 * W:], in_=xt[n, q, :, 35 * W:])
                if n == 0 and q == 0:
                    # bias load is off the first-matmul critical path
                    nc.sync.dma_start(out=bt, in_=bv[:])
                xr = xs.rearrange("p (h w) -> p h w", w=W)
                ot = opool.tile([128, NOUT], i8, tag="ot")
                HN = NOUT // 2
                for half in range(2):
                    banks = range(3 * half, 3 * half + 3)
                    pss = [pspool.tile([128, BANKN], f32, tag=f"ps{i}",
                                       name=f"ps{i}")
                           for i in range(3)]
                    for t in range(9):
                        i, j = divmod(t, 3)
                        for bi, b in enumerate(banks):
                            rhs = xr[:, 9 * b + i: 9 * b + i + 9, j: j + 54]
                            nc.tensor.matmul(pss[bi], lhsT=ws[:, q, t, :],
                                             rhs=rhs, start=(t == 0),
                                             stop=(t == 8),
                                             skip_group_check=True)
                    for bi, b in enumerate(banks):
                        osl = ot[:, BANKN * b: BANKN * (b + 1)]
                        if bi < 2:
                            nc.scalar.activation(out=osl, in_=pss[bi],
                                                 func=Act.Identity,
                                                 bias=bt[:, q], scale=1.0)
                        else:
                            nc.vector.tensor_scalar(out=osl, in0=pss[bi],
                                                    scalar1=bt[:, q],
                                                    scalar2=None, op0=Alu.add)
                    hs = slice(HN * half, HN * (half + 1))
                    nc.sync.dma_start(out=out[n, q, :, hs], in_=ot[:, hs])

    nc.finalize()
    return nc


def kernel(X, W, bias, Werr, Berr, _trace=False):
    X = np.asarray(X, np.float32)
    W = np.asarray(W, np.float32)
    bias = np.asarray(bias, np.float32)
    Werr = np.asarray(Werr, np.float32)
    Berr = np.asarray(Berr, np.float32)

    if "nc" not in _cached:
        _cached["nc"] = _build()
    nc = _cached["nc"]

    Xh = X.astype(np.float16)  # [64, 56, 56, 256]
    w3 = W[..., 0]             # [3, 3, 256]
    we3 = Werr[..., 0]         # [8, 3, 3, 256]

    in_maps = []
    for p in range(POOL):
        xp = Xh[p * NB:(p + 1) * NB].reshape(NB, NPIX, C)
        xp = np.ascontiguousarray(xp.transpose(0, 2, 1)).reshape(NB, NCH, 128, NPIX)

        w_eff = (np.float32(S) * w3 * we3[p]).astype(np.float16)  # [3, 3, 256]
        wdg = np.zeros((NCH, 9, 128, 128), np.float16)
        for q in range(NCH):
            for t in range(9):
                i, j = divmod(t, 3)
                np.fill_diagonal(wdg[q, t], w_eff[i, j, 128 * q:128 * (q + 1)])
        wdg = np.ascontiguousarray(wdg.transpose(2, 0, 1, 3))  # [128,NCH,9,128]

        b_eff = (np.float32(S) * bias * Berr[p]).astype(np.float32)
        bv = np.ascontiguousarray(b_eff.reshape(NCH, 128, 1).transpose(1, 0, 2))
        in_maps.append({"xt": xp, "wdg": wdg, "bv": bv})

    res = run_bass_kernel_spmd(nc, in_maps, core_ids=list(range(POOL)),
                               trace=_trace)
    if _trace:
        _cached["last_result"] = res

    outs = []
    for p in range(POOL):
        o = res.results[p]["out"]  # [NB, NCH, 128, NOUT] int8
        o = np.where(o == -128, np.int8(-127), o).astype(np.float32)
        o = o / np.float32(S)
        o = o.reshape(NB, C, HO, WO).transpose(0, 2, 3, 1)  # NHWC
        outs.append(o)
    return np.ascontiguousarray(np.concatenate(outs, axis=0).astype(np.float32))


# revision 23
# speedup vs baseline: 1.5148x; 1.0061x over previous
"""Trainium2 Bass kernel for nn_DepthWiseConv_AConnect (depthwise 3x3 conv with
per-pool multiplicative weight/bias noise, followed by 8-bit LQuant).

Strategy (8 NeuronCores, data-parallel over the pool axis):
  - Core p handles pool group p: 8 images, Werr[p], Berr[p].
  - Inputs ship as fp16 (products of two 11-bit-mantissa halves are exact in
    the fp32 PSUM accumulator); quantized outputs ship as int8.
  - Host pre-transposes X to channels-major [n, c, h*w] fp16.
  - On device, the depthwise conv is 9 accumulating diagonal matmuls on the
    TensorEngine per PSUM bank tile of 486 output pixels (9 output rows):
    psum[c, pix] += diag(127*w_t)[c] * X^T[c, pix + shift_t].
    Matmuls run tap-outer over 3-bank halves so consecutive instructions
    share the stationary weights.
  - PSUM evict IS the quantization: activation/tensor_scalar compute
    (psum + 127*bias_noisy[c]) with an int8 output dtype.  The hardware
    f32->int8 convert rounds to nearest-even (verified bit-identical to the
    jnp.round magic-constant trick) and saturates to [-128, 127]; the host
    maps the (astronomically rare) -128 back to -127 to match the
    reference's clip-before-scale.  ScalarE drains 2 banks per half and
    VectorE the third, so the post-matmul drain chain is short.
  - Host divides by 127 and transposes back to NHWC.

Performance notes (measured on trn2, core 0):
  - The TensorEngine is the critical path and runs gap-free at full p-state:
    864 matmuls x 486 columns x 0.4167 ns ~= 175 us; LDWEIGHTS pipelines
    behind the column stream.  Columns = out_pixels x 9 taps x C/128 is
    provably minimal for a depthwise conv on the 128-partition PE.  fp8
    DoubleRow was measured to give NO speedup for tap-packing: hardware
    streams the doubled moving rows at 0.5 cyc each (same wall time as one
    fp16 matmul), so a hi/lo-split DR variant ran 301 us despite passing
    the rel-err gate at 6.7e-3.
  - GPSIMD must not touch the data path: a [128, 2916] tensor_scalar costs
    ~43 us there (vs ~1.7 us on VectorE), which made the original version
    DMA-starved and 3.5x slower.
  - The MAGIC bias must be added to fp32 data on-chip; folding it into the
    host-side bias rounds the bias to integers (ulp(1.5*2^23) = 1).
  - Head ~13 us (framework preamble + first loads), tail ~9 us (drain +
    epilogue); run-to-run DVFS variance is ~20%.
"""
import sys

import numpy as np

try:
    import concourse.bacc as bacc_mod
except ImportError:
    sys.path.insert(0, "/opt/trn_rl_repo")
    import concourse.bacc as bacc_mod

import concourse.mybir as mybir
from concourse.tile import TileContext
from concourse.bass_utils import run_bass_kernel_spmd
from contextlib import ExitStack

POOL = 8
NB = 8            # images per pool group (64 / 8)
H = W = 56
HO = WO = 54
C = 256
NCH = 2           # channel chunks of 128
NPIX = H * W      # 3136
NOUT = HO * WO    # 2916
NBANK = 6         # psum bank tiles per plane (6 * 486 = 2916)
BANKN = 486       # output pixels per psum tile (9 rows x 54)
MAGIC = 12582912.0  # 1.5 * 2^23
S = 127.0

f32 = mybir.dt.float32
f16 = mybir.dt.float16
i8 = mybir.dt.int8
Alu = mybir.AluOpType
Act = mybir.ActivationFunctionType

_cached = {}


def _build():
    nc = bacc_mod.Bacc()
    xt = nc.dram_tensor("xt", [NB, NCH, 128, NPIX], f16, kind="ExternalInput")
    wdg = nc.dram_tensor("wdg", [128, NCH, 9, 128], f16, kind="ExternalInput")
    bv = nc.dram_tensor("bv", [128, NCH, 1], f32, kind="ExternalInput")
    out = nc.dram_tensor("out", [NB, NCH, 128, NOUT], i8, kind="ExternalOutput")

    with TileContext(nc) as tc, ExitStack() as ctx:
        consts = ctx.enter_context(tc.tile_pool(name="consts", bufs=1))
        xpool = ctx.enter_context(tc.tile_pool(name="xpool", bufs=3))
        tpool = ctx.enter_context(tc.tile_pool(name="tpool", bufs=2))
        opool = ctx.enter_context(tc.tile_pool(name="opool", bufs=2))
        pspool = ctx.enter_context(tc.tile_pool(name="pspool", bufs=2, space="PSUM"))

        ws = consts.tile([128, NCH, 9, 128], f16)
        # only the q=0 weights gate the first matmul; load the rest after
        # the first image rows
        nc.sync.dma_start(out=ws[:, 0], in_=wdg[:, 0])
        bt = consts.tile([128, NCH, 1], f32)

        for n in range(NB):
            for q in range(NCH):
                xs = xpool.tile([128, NPIX], f16, tag="xs")
                # split the load so the first half's matmuls (input rows
                # 0..34) don't wait for the whole image
                nc.sync.dma_start(out=xs[:, :35 * W], in_=xt[n, q, :, :35 * W])
                nc.sync.dma_start(out=xs[:, 35 * W:], in_=xt[n, q, :, 35 * W:])
                if n == 0 and q == 0:
                    # q=1 weights and bias are off the first-matmul
                    # critical path
                    nc.sync.dma_start(out=ws[:, 1], in_=wdg[:, 1])
                    nc.sync.dma_start(out=bt, in_=bv[:])
                xr = xs.rearrange("p (h w) -> p h w", w=W)
                ot = opool.tile([128, NOUT], i8, tag="ot")
                HN = NOUT // 2
                for half in range(2):
                    banks = range(3 * half, 3 * half + 3)
                    pss = [pspool.tile([128, BANKN], f32, tag=f"ps{i}",
                                       name=f"ps{i}")
                           for i in range(3)]
                    for t in range(9):
                        i, j = divmod(t, 3)
                        for bi, b in enumerate(banks):
                            rhs = xr[:, 9 * b + i: 9 * b + i + 9, j: j + 54]
                            nc.tensor.matmul(pss[bi], lhsT=ws[:, q, t, :],
                                             rhs=rhs, start=(t == 0),
                                             stop=(t == 8),
                                             skip_group_check=True)
                    for bi, b in enumerate(banks):
                        osl = ot[:, BANKN * b: BANKN * (b + 1)]
                        if bi < 2:
                            nc.scalar.activation(out=osl, in_=pss[bi],
                                                 func=Act.Identity,
                                                 bias=bt[:, q], scale=1.0)
                        else:
                            nc.vector.tensor_scalar(out=osl, in0=pss[bi],
                                                    scalar1=bt[:, q],
                                                    scalar2=None, op0=Alu.add)
                    hs = slice(HN * half, HN * (half + 1))
                    nc.sync.dma_start(out=out[n, q, :, hs], in_=ot[:, hs])

    nc.finalize()
    return nc


def kernel(X, W, bias, Werr, Berr, _trace=False):
    X = np.asarray(X, np.float32)
    W = np.asarray(W, np.float32)
    bias = np.asarray(bias, np.float32)
    Werr = np.asarray(Werr, np.float32)
    Berr = np.asarray(Berr, np.float32)

    if "nc" not in _cached:
        _cached["nc"] = _build()
    nc = _cached["nc"]

    Xh = X.astype(np.float16)  # [64, 56, 56, 256]
    w3 = W[..., 0]             # [3, 3, 256]
    we3 = Werr[..., 0]         # [8, 3, 3, 256]

    in_maps = []
    for p in range(POOL):
        xp = Xh[p * NB:(p + 1) * NB].reshape(NB, NPIX, C)
        xp = np.ascontiguousarray(xp.transpose(0, 2, 1)).reshape(NB, NCH, 128, NPIX)

        w_eff = (np.float32(S) * w3 * we3[p]).astype(np.float16)  # [3, 3, 256]
        wdg = np.zeros((NCH, 9, 128, 128), np.float16)
        for q in range(NCH):
            for t in range(9):
                i, j = divmod(t, 3)
                np.fill_diagonal(wdg[q, t], w_eff[i, j, 128 * q:128 * (q + 1)])
        wdg = np.ascontiguousarray(wdg.transpose(2, 0, 1, 3))  # [128,NCH,9,128]

        b_eff = (np.float32(S) * bias * Berr[p]).astype(np.float32)
        bv = np.ascontiguousarray(b_eff.reshape(NCH, 128, 1).transpose(1, 0, 2))
        in_maps.append({"xt": xp, "wdg": wdg, "bv": bv})

    res = run_bass_kernel_spmd(nc, in_maps, core_ids=list(range(POOL)),
                               trace=_trace)
    if _trace:
        _cached["last_result"] = res

    outs = []
    for p in range(POOL):
        o = res.results[p]["out"]  # [NB, NCH, 128, NOUT] int8
        o = np.where(o == -128, np.int8(-127), o).astype(np.float32)
        o = o / np.float32(S)
        o = o.reshape(NB, C, HO, WO).transpose(0, 2, 3, 1)  # NHWC
        outs.append(o)
    return np.ascontiguousarray(np.concatenate(outs, axis=0).astype(np.float32))


# revision 24
# speedup vs baseline: 1.5288x; 1.0092x over previous
"""Trainium2 Bass kernel for nn_DepthWiseConv_AConnect (depthwise 3x3 conv with
per-pool multiplicative weight/bias noise, followed by 8-bit LQuant).

Strategy (8 NeuronCores, data-parallel over the pool axis):
  - Core p handles pool group p: 8 images, Werr[p], Berr[p].
  - Inputs ship as fp16 (products of two 11-bit-mantissa halves are exact in
    the fp32 PSUM accumulator); quantized outputs ship as int8.
  - Host pre-transposes X to channels-major [n, c, h*w] fp16.
  - On device, the depthwise conv is 9 accumulating diagonal matmuls on the
    TensorEngine per PSUM bank tile of 486 output pixels (9 output rows):
    psum[c, pix] += diag(127*w_t)[c] * X^T[c, pix + shift_t].
    Matmuls run tap-outer over 3-bank halves so consecutive instructions
    share the stationary weights.
  - PSUM evict IS the quantization: activation/tensor_scalar compute
    (psum + 127*bias_noisy[c]) with an int8 output dtype.  The hardware
    f32->int8 convert rounds to nearest-even (verified bit-identical to the
    jnp.round magic-constant trick) and saturates to [-128, 127]; the host
    maps the (astronomically rare) -128 back to -127 to match the
    reference's clip-before-scale.  ScalarE drains 2 banks per half and
    VectorE the third, so the post-matmul drain chain is short.
  - Host divides by 127 and transposes back to NHWC.

Performance notes (measured on trn2, core 0):
  - The TensorEngine is the critical path and runs gap-free at full p-state:
    864 matmuls x 486 columns x 0.4167 ns ~= 175 us; LDWEIGHTS pipelines
    behind the column stream.  Columns = out_pixels x 9 taps x C/128 is
    provably minimal for a depthwise conv on the 128-partition PE.  fp8
    DoubleRow was measured to give NO speedup for tap-packing: hardware
    streams the doubled moving rows at 0.5 cyc each (same wall time as one
    fp16 matmul), so a hi/lo-split DR variant ran 301 us despite passing
    the rel-err gate at 6.7e-3.
  - GPSIMD must not touch the data path: a [128, 2916] tensor_scalar costs
    ~43 us there (vs ~1.7 us on VectorE), which made the original version
    DMA-starved and 3.5x slower.
  - The MAGIC bias must be added to fp32 data on-chip; folding it into the
    host-side bias rounds the bias to integers (ulp(1.5*2^23) = 1).
  - Head ~13 us (framework preamble + first loads), tail ~9 us (drain +
    epilogue); run-to-run DVFS variance is ~20%.
"""
import sys

import numpy as np

try:
    import concourse.bacc as bacc_mod
except ImportError:
    sys.path.insert(0, "/opt/trn_rl_repo")
    import concourse.bacc as bacc_mod

import concourse.mybir as mybir
from concourse.tile import TileContext
from concourse.bass_utils import run_bass_kernel_spmd
from contextlib import ExitStack

POOL = 8
NB = 8            # images per pool group (64 / 8)
H = W = 56
HO = WO = 54
C = 256
NCH = 2           # channel chunks of 128
NPIX = H * W      # 3136
NOUT = HO * WO    # 2916
NBANK = 6         # psum bank tiles per plane (6 * 486 = 2916)
BANKN = 486       # output pixels per psum tile (9 rows x 54)
MAGIC = 12582912.0  # 1.5 * 2^23
S = 127.0

f32 = mybir.dt.float32
f16 = mybir.dt.float16
i8 = mybir.dt.int8
Alu = mybir.AluOpType
Act = mybir.ActivationFunctionType

_cached = {}


def _build():
    nc = bacc_mod.Bacc()
    xt = nc.dram_tensor("xt", [NB, NCH, 128, NPIX], f16, kind="ExternalInput")
    wdg = nc.dram_tensor("wdg", [128, NCH, 9, 128], f16, kind="ExternalInput")
    bv = nc.dram_tensor("bv", [128, NCH, 1], f32, kind="ExternalInput")
    out = nc.dram_tensor("out", [NB, NCH, 128, NOUT], i8, kind="ExternalOutput")

    with TileContext(nc) as tc, ExitStack() as ctx:
        consts = ctx.enter_context(tc.tile_pool(name="consts", bufs=1))
        xpool = ctx.enter_context(tc.tile_pool(name="xpool", bufs=4))
        tpool = ctx.enter_context(tc.tile_pool(name="tpool", bufs=2))
        opool = ctx.enter_context(tc.tile_pool(name="opool", bufs=3))
        pspool = ctx.enter_context(tc.tile_pool(name="pspool", bufs=2, space="PSUM"))
        scrpool = ctx.enter_context(tc.tile_pool(name="scrpool", bufs=1,
                                                 space="PSUM"))

        ws = consts.tile([128, NCH, 9, 128], f16)
        # only the q=0 weights gate the first matmul; load the rest after
        # the first image rows
        nc.sync.dma_start(out=ws[:, 0], in_=wdg[:, 0])
        bt = consts.tile([128, NCH, 1], f32)

        # Warm up the TensorEngine p-state during the initial DMA wait: the
        # PE needs ~3us of continuous execution to reach 2.4 GHz, so run a
        # chain of small matmuls on zeroed scratch data sized to end just as
        # the first image rows land (~5us after the preamble).  Short 64-col
        # matmuls keep the chain's end-time granularity fine so the real
        # first matmul is delayed by at most ~60ns on a fast-DMA run.
        dummy = consts.tile([128, 128], f16)
        nc.vector.memset(dummy, 0.0)
        sps = scrpool.tile([128, 64], f32)
        for _ in range(80):
            nc.tensor.matmul(sps, lhsT=dummy, rhs=dummy[:, :64], start=True,
                             stop=True, skip_group_check=True)

        for n in range(NB):
            for q in range(NCH):
                xs = xpool.tile([128, NPIX], f16, tag="xs")
                # split the load so the first half's matmuls (input rows
                # 0..34) don't wait for the whole image
                nc.sync.dma_start(out=xs[:, :35 * W], in_=xt[n, q, :, :35 * W])
                nc.sync.dma_start(out=xs[:, 35 * W:], in_=xt[n, q, :, 35 * W:])
                if n == 0 and q == 0:
                    # q=1 weights and bias are off the first-matmul
                    # critical path
                    nc.sync.dma_start(out=ws[:, 1], in_=wdg[:, 1])
                    nc.sync.dma_start(out=bt, in_=bv[:])
                xr = xs.rearrange("p (h w) -> p h w", w=W)
                ot = opool.tile([128, NOUT], i8, tag="ot")
                HN = NOUT // 2
                for half in range(2):
                    banks = range(3 * half, 3 * half + 3)
                    pss = [pspool.tile([128, BANKN], f32, tag=f"ps{i}",
                                       name=f"ps{i}")
                           for i in range(3)]
                    for t in range(9):
                        i, j = divmod(t, 3)
                        for bi, b in enumerate(banks):
                            rhs = xr[:, 9 * b + i: 9 * b + i + 9, j: j + 54]
                            nc.tensor.matmul(pss[bi], lhsT=ws[:, q, t, :],
                                             rhs=rhs, start=(t == 0),
                                             stop=(t == 8),
                                             skip_group_check=True)
                    for bi, b in enumerate(banks):
                        osl = ot[:, BANKN * b: BANKN * (b + 1)]
                        if bi < 2:
                            nc.scalar.activation(out=osl, in_=pss[bi],
                                                 func=Act.Identity,
                                                 bias=bt[:, q], scale=1.0)
                        else:
                            nc.vector.tensor_scalar(out=osl, in0=pss[bi],
                                                    scalar1=bt[:, q],
                                                    scalar2=None, op0=Alu.add)
                    hs = slice(HN * half, HN * (half + 1))
                    nc.sync.dma_start(out=out[n, q, :, hs], in_=ot[:, hs])

    nc.finalize()
    return nc


def kernel(X, W, bias, Werr, Berr, _trace=False):
    X = np.asarray(X, np.float32)
    W = np.asarray(W, np.float32)
    bias = np.asarray(bias, np.float32)
    Werr = np.asarray(Werr, np.float32)
    Berr = np.asarray(Berr, np.float32)

    if "nc" not in _cached:
        _cached["nc"] = _build()
    nc = _cached["nc"]

    Xh = X.astype(np.float16)  # [64, 56, 56, 256]
    w3 = W[..., 0]             # [3, 3, 256]
    we3 = Werr[..., 0]         # [8, 3, 3, 256]

    in_maps = []
    for p in range(POOL):
        xp = Xh[p * NB:(p + 1) * NB].reshape(NB, NPIX, C)
        xp = np.ascontiguousarray(xp.transpose(0, 2, 1)).reshape(NB, NCH, 128, NPIX)

        w_eff = (np.float32(S) * w3 * we3[p]).astype(np.float16)  # [3, 3, 256]
        wdg = np.zeros((NCH, 9, 128, 128), np.float16)
        for q in range(NCH):
            for t in range(9):
                i, j = divmod(t, 3)
                np.fill_diagonal(wdg[q, t], w_eff[i, j, 128 * q:128 * (q + 1)])
        wdg = np.ascontiguousarray(wdg.transpose(2, 0, 1, 3))  # [128,NCH,9,128]

        b_eff = (np.float32(S) * bias * Berr[p]).astype(np.float32)
        bv = np.ascontiguousarray(b_eff.reshape(NCH, 128, 1).transpose(1, 0, 2))
        in_maps.append({"xt": xp, "wdg": wdg, "bv": bv})

    res = run_bass_kernel_spmd(nc, in_maps, core_ids=list(range(POOL)),
                               trace=_trace)
    if _trace:
        _cached["last_result"] = res

    outs = []
    for p in range(POOL):
        o = res.results[p]["out"]  # [NB, NCH, 128, NOUT] int8
        o = np.where(o == -128, np.int8(-127), o).astype(np.float32)
        o = o / np.float32(S)
        o = o.reshape(NB, C, HO, WO).transpose(0, 2, 3, 1)  # NHWC
        outs.append(o)
    return np.ascontiguousarray(np.concatenate(outs, axis=0).astype(np.float32))
